# revision 9
# baseline (speedup 1.0000x reference)
"""Trainium2 Bass kernel for InterventionAwareStructure loss.

loss = sum_b,i,d A[b,i,d] * mask[regimes[b], d] / count   (scalar)

Data-parallel over batch across 8 NeuronCores. Each core:
  - streams its A shard [32, 512, 512] from HBM in 2 MB chunks on the
    SP HWDGE ring (fp32 bits re-tagged as fp32r via a dram-tensor
    bitcast, so no SWDGE cast path and no Q7 descriptor-emission
    latency; 16 KB descriptor lines run the SDMA engines at line rate),
  - TensorE reduces each chunk over the source axis i with one-hot
    block stationaries (1 cycle/row in fp32r), accumulating ALL chunks
    into a single [32, 512] PSUM tile; the last two batch items ride
    in separate 1 MB chunks so the tail matmul backlog stays short,
  - one final VectorE copy moves the PSUM colsums to SBUF and they are
    DMA'd out as [32, 512]; the host does the tiny mask dot, the
    cross-core sum, and the divide by count.

The mask gather (256x512), the mask dot, and the final scalar
reduction are all done on host; they are negligible next to the
256 MB stream of A.
"""

import numpy as np

import concourse.bass as bass
import concourse.tile as tile
from concourse import bacc, mybir
from concourse.bass_utils import run_bass_kernel_spmd

INTERVENTION_STRENGTH = 1.0

N_CORES = 8
B, N_REGIMES, D = 256, 16, 512
B_SH = B // N_CORES          # 32 batch items per core
NBIG = B_SH // 2 - 1         # 15 full 2 MB chunks (2 batch items each)
FREE2 = 2 * D * D // 128     # 4096 f32 per partition per 2 MB chunk
FREE1 = D * D // 128         # 2048 f32 per partition per 1 MB chunk

_CACHED_NC = None


def _build_nc() -> bass.Bass:
    nc = bacc.Bacc()
    f32 = mybir.dt.float32
    f32r = mybir.dt.float32r

    # fp32 bits, tagged fp32r so HWDGE can move them without a cast.
    a = nc.dram_tensor("a", [B_SH, D, D], f32, kind="ExternalInput").bitcast(f32r)
    out = nc.dram_tensor("out", [B_SH, D], f32, kind="ExternalOutput")

    # One-hot block stationaries, one aligned [128, 32] block per chunk.
    # 2 MB chunk g holds batch items (2g, 2g+1) split across partition
    # halves: W[p, g*32 + q] = 1 iff q == 2g + p//64.  The two tail 1 MB
    # chunks hold one batch item on all 128 partitions:
    # W[p, (NBIG+t)*32 + q] = 1 iff q == 30 + t.
    w_np = np.zeros((128, (NBIG + 2) * 32), dtype=np.float32)
    for g in range(NBIG):
        w_np[np.arange(128), g * 32 + 2 * g + np.arange(128) // 64] = 1.0
    for t in range(2):
        w_np[:, (NBIG + t) * 32 + 30 + t] = 1.0
    wsel = nc.inline_tensor(w_np, "wsel").bitcast(f32r)

    # 2 MB chunk g of batches (2g, 2g+1) -> SBUF [128, FREE2]: partition
    # p = (gb * 64 + ih) holds rows i = ih*8 + il of batch 2g+gb; free
    # axis = (il, d) with 16 KB contiguous per partition line.
    a_view2 = a.rearrange(
        "(ng gb) (ih il) d -> ng (gb ih) (il d)", ng=B_SH // 2, ih=64
    )
    # 1 MB tail chunk of batch b -> SBUF [128, FREE1]: partition ih
    # holds rows i = ih*4 + il; free axis = (il, d).
    a_view1 = a.rearrange("b (ih il) d -> b ih (il d)", ih=128)

    with tile.TileContext(nc) as tc:
        with (
            tc.tile_pool(name="big", bufs=8) as big_pool,
            tc.tile_pool(name="tail", bufs=2) as tail_pool,
            tc.tile_pool(name="small", bufs=1) as small_pool,
            tc.tile_pool(name="psum", bufs=1, space="PSUM") as psum_pool,
        ):
            w_t = small_pool.tile([128, (NBIG + 2) * 32], f32r)
            nc.scalar.dma_start(w_t[:], wsel[:])

            big_tiles = []
            for g in range(NBIG):
                a_t = big_pool.tile([128, FREE2], f32r, tag="a")
                nc.sync.dma_start(a_t[:], a_view2[g])
                big_tiles.append(a_t)
            tail_tiles = []
            for t in range(2):
                a_t = tail_pool.tile([128, FREE1], f32r, tag="t")
                if t == 1:
                    # Split the final 1 MB so its first matmuls overlap
                    # the stream of its second half.
                    half = FREE1 // 2
                    nc.sync.dma_start(a_t[:, :half], a_view1[30 + t][:, :half])
                    nc.sync.dma_start(a_t[:, half:], a_view1[30 + t][:, half:])
                else:
                    nc.sync.dma_start(a_t[:], a_view1[30 + t])
                tail_tiles.append(a_t)

            ps = psum_pool.tile([B_SH, D], f32)
            for g in range(NBIG):
                a_t = big_tiles[g]
                w_g = w_t[:, g * 32:(g + 1) * 32]
                for j in range(FREE2 // D):
                    nc.tensor.matmul(
                        ps[:], w_g, a_t[:, j * D:(j + 1) * D],
                        start=(g == 0 and j == 0), stop=False,
                    )
            for t in range(2):
                a_t = tail_tiles[t]
                w_g = w_t[:, (NBIG + t) * 32:(NBIG + t + 1) * 32]
                for j in range(FREE1 // D):
                    nc.tensor.matmul(
                        ps[:], w_g, a_t[:, j * D:(j + 1) * D],
                        start=False, stop=(t == 1 and j == FREE1 // D - 1),
                    )

            o_t = small_pool.tile([B_SH, D], f32)
            nc.vector.tensor_copy(o_t[:], ps[:])
            nc.sync.dma_start(out[:], o_t[:])

    nc.finalize()
    return nc


def _get_nc() -> bass.Bass:
    global _CACHED_NC
    if _CACHED_NC is None:
        _CACHED_NC = _build_nc()
    return _CACHED_NC


def _run(a_shards, **run_kwargs):
    nc = _get_nc()
    in_maps = [{"a": np.ascontiguousarray(a_shards[c])} for c in range(N_CORES)]
    return run_bass_kernel_spmd(nc, in_maps, list(range(N_CORES)), **run_kwargs)


def kernel(A_per_env, intervention_mask, regimes, _run_kwargs=None):
    A_per_env = np.asarray(A_per_env, dtype=np.float32)
    intervention_mask = np.asarray(intervention_mask, dtype=np.float32)
    regs = np.asarray(regimes).astype(np.int64)

    n_regimes = intervention_mask.shape[0]
    valid = regs < n_regimes
    e = np.clip(regs, 0, n_regimes - 1)
    masks = intervention_mask[e] * valid[:, None].astype(np.float32)  # [B, D]

    a_shards = [A_per_env[c * B_SH:(c + 1) * B_SH] for c in range(N_CORES)]

    res = _run(a_shards, **(_run_kwargs or {}))
    num = np.float64(0.0)
    for c in range(N_CORES):
        colsums = res.results[c]["out"].astype(np.float64)        # [32, 512]
        num += (colsums * masks[c * B_SH:(c + 1) * B_SH]).sum()

    count = masks.astype(np.float64).sum()
    loss = num / count if count > 0 else num
    out = np.asarray(INTERVENTION_STRENGTH * loss, dtype=np.float32)
    if _run_kwargs is not None:
        return out, res
    return out


# revision 12
# speedup vs baseline: 1.0201x; 1.0201x over previous
"""Trainium2 Bass kernel for InterventionAwareStructure loss.

loss = sum_b,i,d A[b,i,d] * mask[regimes[b], d] / count   (scalar)

Data-parallel over batch across 8 NeuronCores. Each core:
  - streams its A shard [32, 512, 512] from HBM in 1 MB chunks on the
    SP HWDGE ring (fp32 bits re-tagged as fp32r via a dram-tensor
    bitcast, so no SWDGE cast path and no Q7 descriptor-emission
    latency),
  - TensorE reduces each chunk over the source axis i with a one-hot
    stationary column (1 cycle/row in fp32r), accumulating ALL 128
    matmuls into a single [32, 512] PSUM tile.  The one-hot stationary
    table is synthesized on the otherwise-idle GpSimd engine
    (memset + affine_select), so no weight bytes ride the HBM stream,
  - one final VectorE copy moves the PSUM colsums to SBUF and they are
    DMA'd out as [32, 512] on the idle ACT HWDGE ring; the host does
    the tiny mask dot, the cross-core sum, and the divide by count.

The mask gather (256x512), the mask dot, and the final scalar
reduction are all done on host; they are negligible next to the
256 MB stream of A.
"""

import numpy as np

import concourse.bass as bass
import concourse.tile as tile
from concourse import bacc, mybir
from concourse.bass_utils import run_bass_kernel_spmd

INTERVENTION_STRENGTH = 1.0

N_CORES = 8
B, N_REGIMES, D = 256, 16, 512
B_SH = B // N_CORES          # 32 batch items per core = 32 x 1 MB chunks
FREE = D * D // 128          # 2048 f32 per partition per 1 MB chunk
NMM = FREE // D              # 4 matmuls of free-dim 512 per chunk

_CACHED_NC = None


def _build_nc() -> bass.Bass:
    nc = bacc.Bacc()
    f32 = mybir.dt.float32
    f32r = mybir.dt.float32r

    # fp32 bits, tagged fp32r so HWDGE can move them without a cast.
    a = nc.dram_tensor("a", [B_SH, D, D], f32, kind="ExternalInput").bitcast(f32r)
    out = nc.dram_tensor("out", [B_SH, D], f32, kind="ExternalOutput")

    # Chunk b -> SBUF tile [128, FREE]: partition ih holds source rows
    # i = ih*NMM + il of batch item b; free axis = (il, d); each
    # partition line is 8 KB contiguous in DRAM.
    a_view = a.rearrange("b (ih il) d -> b ih (il d)", ih=128)

    with tile.TileContext(nc) as tc:
        with (
            tc.tile_pool(name="big", bufs=16) as big_pool,
            tc.tile_pool(name="small", bufs=1) as small_pool,
            tc.tile_pool(name="psum", bufs=1, space="PSUM") as psum_pool,
        ):
            # One-hot stationary table, built on-chip: W[p, 33*u + v] = 1
            # iff v == 0, i.e. ones exactly at flat columns c = 33*u.
            # Chunk b uses the 32-aligned slice [:, 32b : 32b+32], whose
            # single one sits at relative column 33b - 32b = b, so its
            # matmuls add colsum(A_b) into row b of the shared PSUM tile.
            w_f = small_pool.tile([128, 33 * B_SH], f32)
            nc.gpsimd.memset(w_f[:], 1.0)
            nc.gpsimd.affine_select(
                out=w_f[:],
                in_=w_f[:],
                pattern=[[0, B_SH], [1, 33]],
                compare_op=mybir.AluOpType.is_equal,
                fill=0.0,
                base=0,
                channel_multiplier=0,
            )
            # The BIR verifier wants fp32r matmul weights produced by an
            # op whose output dtype is fp32r; a DVE copy does the re-tag.
            w_t = small_pool.tile([128, 33 * B_SH], f32r)
            nc.vector.tensor_copy(w_t[:], w_f[:])

            a_tiles = []
            for b in range(B_SH):
                a_t = big_pool.tile([128, FREE], f32r, tag="a")
                if b == B_SH - 1:
                    # Split the last chunk so the tail matmuls start on
                    # its first half while the second half streams.
                    half = FREE // 2
                    nc.sync.dma_start(a_t[:, :half], a_view[b][:, :half])
                    nc.sync.dma_start(a_t[:, half:], a_view[b][:, half:])
                else:
                    nc.sync.dma_start(a_t[:], a_view[b])
                a_tiles.append(a_t)

            ps = psum_pool.tile([B_SH, D], f32)
            for b in range(B_SH):
                a_t = a_tiles[b]
                w_b = w_t[:, 32 * b:32 * b + 32]
                for j in range(NMM):
                    nc.tensor.matmul(
                        ps[:], w_b, a_t[:, j * D:(j + 1) * D],
                        start=(b == 0 and j == 0),
                        stop=(b == B_SH - 1 and j == NMM - 1),
                    )

            o_t = small_pool.tile([B_SH, D], f32)
            nc.vector.tensor_copy(o_t[:], ps[:])
            nc.scalar.dma_start(out[:], o_t[:])

    nc.finalize()
    return nc


def _get_nc() -> bass.Bass:
    global _CACHED_NC
    if _CACHED_NC is None:
        _CACHED_NC = _build_nc()
    return _CACHED_NC


def _run(a_shards, **run_kwargs):
    nc = _get_nc()
    in_maps = [{"a": np.ascontiguousarray(a_shards[c])} for c in range(N_CORES)]
    return run_bass_kernel_spmd(nc, in_maps, list(range(N_CORES)), **run_kwargs)


def kernel(A_per_env, intervention_mask, regimes, _run_kwargs=None):
    A_per_env = np.asarray(A_per_env, dtype=np.float32)
    intervention_mask = np.asarray(intervention_mask, dtype=np.float32)
    regs = np.asarray(regimes).astype(np.int64)

    n_regimes = intervention_mask.shape[0]
    valid = regs < n_regimes
    e = np.clip(regs, 0, n_regimes - 1)
    masks = intervention_mask[e] * valid[:, None].astype(np.float32)  # [B, D]

    a_shards = [A_per_env[c * B_SH:(c + 1) * B_SH] for c in range(N_CORES)]

    res = _run(a_shards, **(_run_kwargs or {}))
    num = np.float64(0.0)
    for c in range(N_CORES):
        colsums = res.results[c]["out"].astype(np.float64)        # [32, 512]
        num += (colsums * masks[c * B_SH:(c + 1) * B_SH]).sum()

    count = masks.astype(np.float64).sum()
    loss = num / count if count > 0 else num
    out = np.asarray(INTERVENTION_STRENGTH * loss, dtype=np.float32)
    if _run_kwargs is not None:
        return out, res
    return out
